# revision 47
# baseline (speedup 1.0000x reference)
"""Trainium2 Bass kernel for nn_Block_54382875902076 (dense transformer block).

Reference computation (B=4, S=2048, E=512, H=8, D=64, fp32):
    res = x
    h   = LN1(x)                      (no bias, eps=1e-6)
    h   = res + Attn(h)               (causal, wo1 [H,D,E] then wo2 [E,E])
    h   = LN2(h)
    out = res + gelu(h @ w1) @ w2     (NOTE: res = block input, both residuals)

Sharding (8 cores): core c = (batch b = c//2, head-group g = c%2).
Each core computes LN1 + QKV for its 4 heads over the full sequence,
exact-causal attention, the wo1 partial projection in [E, token]
orientation, then pair-wise ReduceScatters sum the two head-groups'
partials and hand each core half of every block's rows (already
E-major, no transposes needed) for wo2 + LN2 + MLP.

Schedule (HW-measured ~281 us, from a 336 us starting point):
 - the attention exp stream on the ACT engine is the spine (~1 us per
   128x1024 chunk; diagonal chunks exp/mask/AV only the causally-valid
   column range); score matmuls run one chunk ahead of the AV matmuls
   so the PE never waits on the exp
 - wo1/RS work drains at the front of the next attention block and
   post-RS wo2/LN2/transpose work drains in its last chunks (the RS
   itself takes 10-20 us), one piece per chunk in the PE's idle slots
 - a full-size warmup ReduceScatter pays the collective channel setup
   (~25 us) off the critical path AND pins the channel to the fast
   hardware-descriptor mode (a small warmup pins it to the slow
   software path: 21 GB/s vs 50 GB/s measured)
 - DMA queues are balanced so nothing phase-critical queues behind a
   blocked wait: bulky w1/w2/xq loads issue after the QKV section,
   masks ride the scalar ring, out-stores ride gpsimd (sync may be
   blocked waiting on RS(3) for the last o1rsb load)
 - LN2 uses ACT's ln/exp rsqrt for tiles 0-5 (before the single gelu
   table switch) and a DVE bit-trick rsqrt (magic seed + 2 Newton
   steps) for the RS(3)-gated tiles 6-7, so the ACT table set never
   thrashes between exp and gelu
 - fp8 DoubleRow for the MLP was tried and REVERTED: e4m3's 3-bit
   mantissa costs ~3e-2 rel L2 on this problem (gate is 2e-2), even
   with range-normalizing scales; numpy-simulated error matched HW
   exactly, so the quantization itself is the floor, not a bug
"""

import functools
import sys
from collections import deque

import numpy as np

for _p in ("/opt/trn_rl_repo", "/root/.axon_site/_ro/trn_rl_repo"):
    if _p not in sys.path:
        sys.path.append(_p)

import ml_dtypes  # noqa: E402
import concourse.bass as bass  # noqa: E402
import concourse.tile as tile  # noqa: E402
from concourse import bacc, mybir  # noqa: E402
from concourse.bass_utils import run_bass_kernel_spmd  # noqa: E402

_ALLOWED_ACT_SETS = {"natural_log_exp_and_others", "gelu_apprx_tanh_and_others"}
_orig_get_act_tables = bacc.get_activation_tables


def _filtered_act_tables(module_arch):
    tabs = _orig_get_act_tables(module_arch)
    return {
        name: (funcs if name in _ALLOWED_ACT_SETS else set())
        for name, funcs in tabs.items()
    }


bacc.get_activation_tables = _filtered_act_tables

F32 = mybir.dt.float32
I32 = mybir.dt.int32
BF16 = mybir.dt.bfloat16
F8 = mybir.dt.float8e4
DR = mybir.MatmulPerfMode.DoubleRow
AF = mybir.ActivationFunctionType
ALU = mybir.AluOpType

B, S, E, H, D = 4, 2048, 512, 8, 64
HG = H // 2            # heads per core
SQ = S // 2            # rows per core after reduce-scatter
NT = S // 128          # 16 token tiles (full seq)
NTQ = SQ // 128        # 8 token tiles (own half)
QTS = S // 512         # 4 q-tiles of 512 for attention
RG_PAIRS = [[0, 1], [2, 3], [4, 5], [6, 7]]


def _build_graph():
    nc = bacc.Bacc("TRN2", target_bir_lowering=False, debug=False, num_devices=8)

    xf = nc.declare_dram_parameter("xf", [128, NT, E], BF16, isOutput=False)
    xq = nc.declare_dram_parameter("xq", [128, NTQ, E], BF16, isOutput=False)
    wq = nc.declare_dram_parameter("wq", [128, 4, HG * D], BF16, isOutput=False)
    wk = nc.declare_dram_parameter("wk", [128, 4, HG * D], BF16, isOutput=False)
    wv = nc.declare_dram_parameter("wv", [128, 4, HG * D], BF16, isOutput=False)
    wo1 = nc.declare_dram_parameter("wo1", [128, 2, E], BF16, isOutput=False)
    wo2 = nc.declare_dram_parameter("wo2", [128, 4, E], BF16, isOutput=False)
    w1 = nc.declare_dram_parameter("w1", [128, 4, 4 * E], BF16, isOutput=False)
    w2 = nc.declare_dram_parameter("w2", [128, 16, E], BF16, isOutput=False)
    masks = nc.declare_dram_parameter("masks", [128, 4, 512], BF16, isOutput=False)
    out = nc.declare_dram_parameter("out", [SQ, E], F32, isOutput=True)

    with tile.TileContext(nc) as tc:
        with (
            tc.tile_pool(name="consts", bufs=1) as consts,
            tc.tile_pool(name="acts", bufs=1) as acts,
            tc.tile_pool(name="work", bufs=3) as work,
            tc.tile_pool(name="big2", bufs=2) as big2,
            tc.tile_pool(name="stats", bufs=6) as stats,
            tc.tile_pool(name="den", bufs=2) as den,
            tc.tile_pool(name="lnw", bufs=5) as lnw,
            tc.tile_pool(name="expp", bufs=3) as expp,
            tc.tile_pool(name="psA", bufs=2, space="PSUM") as psA,
            tc.tile_pool(name="psB", bufs=2, space="PSUM") as psB,
            tc.tile_pool(name="psC", bufs=2, space="PSUM") as psC,
            tc.tile_pool(name="dram", bufs=1, space="DRAM") as dram,
        ):
            # warmup buffers shaped exactly like the real RS: the first
            # collective fixes the channel's descriptor mode, and a tiny
            # warmup would pin it to the slow software-descriptor path
            warm_in = dram.tile([2 * E * 256], BF16, name="warm_in")
            warm_out = dram.tile([E * 256], BF16, name="warm_out")
            # flat per-block buffers: a fully-contiguous 1D AP keeps the
            # collective's DMA on the fast hardware-descriptor path
            o1T_d = dram.tile([QTS, 2 * E * 256], BF16, name="o1T_d")
            o1rT_d = dram.tile([QTS, E * 256], BF16, name="o1rT_d")

            # ---- constants / weights (contiguous loads, gpsimd queue) ----
            eps_t = consts.tile([128, 1], F32)
            nc.vector.memset(eps_t, 1e-6)
            magic_t = consts.tile([128, 1], I32, tag="magic")
            nc.vector.memset(magic_t, 0x5F3759DF)
            ident = consts.tile([128, 128], BF16)
            from concourse.masks import make_identity
            make_identity(nc, ident[:])

            def load_const(shape, src, tag, dtype=BF16):
                t = consts.tile(shape, dtype, tag=tag)
                nc.gpsimd.dma_start(t[:], src[:])
                return t

            wq_sb = load_const([128, 4, HG * D], wq, "wq_sb")
            wk_sb = load_const([128, 4, HG * D], wk, "wk_sb")
            wv_sb = load_const([128, 4, HG * D], wv, "wv_sb")
            wo1_sb = load_const([128, 2, E], wo1, "wo1_sb")
            wo2_sb = load_const([128, 4, E], wo2, "wo2_sb")
            masks_sb = consts.tile([128, 4, 512], BF16, tag="masks_sb")
            nc.scalar.dma_start(masks_sb[:], masks[:])
            xq_sb = acts.tile([128, NTQ, E], BF16)

            def layernorm_tile(src_ap, dst_tile):
                """dst (bf16) = (src - mean) * rsqrt(var + eps); ACT rsqrt."""
                st6 = stats.tile([128, 6], F32, tag="st6")
                nc.vector.bn_stats(st6[:], src_ap)
                mv = stats.tile([128, 2], F32, tag="mv")
                nc.vector.bn_aggr(mv[:], st6[:])
                lnv = stats.tile([128, 1], F32, tag="lnv")
                nc.scalar.activation(lnv[:], mv[:, 1:2], AF.Ln, bias=eps_t[:])
                rsig = stats.tile([128, 1], F32, tag="rsig")
                nc.scalar.activation(rsig[:], lnv[:], AF.Exp, scale=-0.5)
                nc.vector.tensor_scalar(
                    dst_tile[:], src_ap, mv[:, 0:1], rsig[:],
                    op0=ALU.subtract, op1=ALU.mult,
                )

            def layernorm_tile_dve(src_ap, dst_tile):
                """Same, but rsqrt on DVE (bit-trick seed + 2 Newton steps)
                so the ACT table set is untouched (no exp/gelu thrash)."""
                st6 = stats.tile([128, 6], F32, tag="st6")
                nc.vector.bn_stats(st6[:], src_ap)
                mv = stats.tile([128, 2], F32, tag="mv")
                nc.vector.bn_aggr(mv[:], st6[:])
                v = mv[:, 1:2]
                y = stats.tile([128, 1], F32, tag="yrs")
                iv = stats.tile([128, 1], I32, tag="ivrs")
                nc.vector.tensor_scalar(
                    iv[:], v.bitcast(I32), 1, None, op0=ALU.arith_shift_right
                )
                nc.vector.tensor_tensor(
                    y.bitcast(I32)[:], magic_t[:], iv[:], op=ALU.subtract
                )
                for it in range(2):
                    y2 = stats.tile([128, 1], F32, tag="y2rs")
                    nc.vector.tensor_tensor(y2[:], y[:], y[:], op=ALU.mult)
                    t = stats.tile([128, 1], F32, tag="trs")
                    nc.vector.tensor_tensor(t[:], y2[:], v, op=ALU.mult)
                    t2 = stats.tile([128, 1], F32, tag="t2rs")
                    nc.vector.tensor_scalar(
                        t2[:], t[:], -0.5, 1.5, op0=ALU.mult, op1=ALU.add
                    )
                    yn = stats.tile([128, 1], F32, tag="ynrs")
                    nc.vector.tensor_tensor(yn[:], y[:], t2[:], op=ALU.mult)
                    y = yn
                nc.vector.tensor_scalar(
                    dst_tile[:], src_ap, mv[:, 0:1], y[:],
                    op0=ALU.subtract, op1=ALU.mult,
                )

            # ---- LN1-folded QKV from host-transposed x -------------------
            # K = rsig*(wk^T x - mu*colsum(wk)) etc: the projections consume
            # xT directly (no h1 transposes); the mean correction is a rank-1
            # accumulate and rsig rides a broadcast multiply (K/Q) or the ACT
            # copy's per-partition scale (V)
            # ---- LN1 + per-block transpose + QKV -------------------------
            xfsb = consts.tile([128, NT, E], BF16, tag="xfsb")
            for _st in range(QTS):
                nc.sync.dma_start(
                    xfsb[:, 4 * _st:4 * _st + 4, :], xf[:, 4 * _st:4 * _st + 4, :]
                )
            h1T = acts.tile([128, 4, S], BF16)
            KT = acts.tile([128, 2, S], BF16)
            QT = acts.tile([128, 2, S], BF16)
            V65 = acts.tile([128, NT, HG, D + 1], BF16)
            nc.vector.memset(V65[:, :, :, D:D + 1], 1.0)
            for st in range(4):
                h1ts0 = []
                for t in range(4 * st, 4 * st + 4):
                    h1t = lnw.tile([128, E], BF16, tag="lnt", name=f"h1t{t}")
                    layernorm_tile(xfsb[:, t, :], h1t)
                    h1ts0.append(h1t)
                for lt in range(4):
                    for ko in range(4):
                        psT = psC.tile([128, 128], BF16, tag="psC",
                                       name=f"psH{st}_{lt}_{ko}")
                        nc.tensor.transpose(
                            psT[:], h1ts0[lt][:, ko * 128:(ko + 1) * 128], ident[:]
                        )
                        nc.vector.tensor_copy(
                            h1T[:, ko, st * 512 + lt * 128:st * 512 + (lt + 1) * 128],
                            psT[:],
                        )
                sl = slice(st * 512, (st + 1) * 512)
                for mi in range(2):
                    for dst, w_sb in ((KT, wk_sb), (QT, wq_sb)):
                        ps = psC.tile([128, 512], F32, tag="psC")
                        for ko in range(4):
                            nc.tensor.matmul(
                                ps[:],
                                lhsT=w_sb[:, ko, mi * 128:(mi + 1) * 128],
                                rhs=h1T[:, ko, sl],
                                start=(ko == 0), stop=(ko == 3),
                            )
                        # ACT is idle in this phase; the PSUM->SBUF casts are
                        # the DVE's biggest load here, so move them over
                        nc.scalar.copy(dst[:, mi, sl], ps[:])
                for tt in range(4 * st, 4 * st + 4):
                    ps = psC.tile([128, 512], F32, tag="psC")
                    for ko in range(4):
                        nc.tensor.matmul(
                            ps[:, 0:HG * D],
                            lhsT=h1T[:, ko, tt * 128:(tt + 1) * 128],
                            rhs=wv_sb[:, ko, :],
                            start=(ko == 0), stop=(ko == 3),
                        )
                    nc.scalar.copy(
                        V65[:, tt, :, 0:D],
                        ps[:, 0:HG * D].rearrange("p (h d) -> p h d", h=HG),
                    )

            # ---- causal attention spine + injected wo1/RS/wo2/LN2 --------
            attnT = acts.tile([128, 2, S], BF16)
            h2T = acts.tile([128, 4, SQ], BF16)
            inject_q = deque()    # wo1 pieces: drained front-of-block
            inject_q2 = deque()   # post-RS pieces: drained in late chunks

            def attention_block(qt):
                ext = 4 * (qt + 1)           # causal extent in 128-chunks
                qsl = slice(qt * 512, (qt + 1) * 512)
                for a in range(2):           # local head pairs (2a, 2a+1)
                    avA = psB.tile([D + 1, 512], F32, tag="psB")
                    avB = psB.tile([D + 1, 512], F32, tag="psB")

                    def scores(c):
                        # the two 64-row matmuls auto-pack into disjoint PE
                        # row-bands (tile_position from base_partition) and
                        # stream concurrently
                        sp = psA.tile([128, 1024], F32, tag="psA")
                        nc.tensor.matmul(
                            sp[:, 0:512],
                            lhsT=KT[0:64, a, c * 128:(c + 1) * 128],
                            rhs=QT[0:64, a, qsl],
                            start=True, stop=True,
                        )
                        nc.tensor.matmul(
                            sp[:, 512:1024],
                            lhsT=KT[64:128, a, c * 128:(c + 1) * 128],
                            rhs=QT[64:128, a, qsl],
                            start=True, stop=True,
                        )
                        return sp

                    sps = {0: scores(0)}
                    for c in range(ext):
                        sp = sps.pop(c)
                        j = c - 4 * qt
                        # diagonal chunk j: columns < 128j have no valid keys
                        # in this chunk, so exp/mask/AV skip them entirely
                        lo = 128 * j if j > 0 else 0
                        ex = expp.tile([128, 2, 512], BF16, tag="ex")
                        nc.scalar.activation(
                            ex[:, :, lo:512],
                            sp.rearrange("p (h f) -> p h f", h=2)[:, :, lo:512],
                            AF.Exp, scale=D ** -0.5,
                        )
                        if c + 1 < ext:      # scores one chunk ahead of AV
                            sps[c + 1] = scores(c + 1)
                        if j >= 0:           # diagonal chunk: causal mask
                            for half in range(2):
                                exh = ex[:, half, lo:512]
                                nc.vector.tensor_mul(
                                    exh, exh, masks_sb[:, j, lo:512]
                                )
                        nc.tensor.matmul(
                            avA[:, lo:512], lhsT=V65[:, c, 2 * a, :],
                            rhs=ex[:, 0, lo:512],
                            start=(c == 0), stop=(c == ext - 1),
                            skip_group_check=True,
                        )
                        nc.tensor.matmul(
                            avB[:, lo:512], lhsT=V65[:, c, 2 * a + 1, :],
                            rhs=ex[:, 1, lo:512],
                            start=(c == 0), stop=(c == ext - 1),
                            skip_group_check=True,
                        )
                        if a == 1 and c >= ext - 8 and inject_q2:
                            inject_q2.popleft()()
                        elif inject_q:
                            inject_q.popleft()()
                    for hh, av in ((2 * a, avA), (2 * a + 1, avB)):
                        # softmax denominator rides as the ones-row (row 64);
                        # reciprocal runs [8,64] (DVE recip cost scales with
                        # free size per lane) via a DMA scatter/gather pair
                        avs = work.tile([D + 1, 512], F32, tag="avs")
                        nc.vector.tensor_copy(avs[:], av[:])
                        d4 = den.tile([8, 64], F32, tag="d4")
                        nc.gpsimd.dma_start(
                            d4[:], avs[D:D + 1, :].rearrange("o (p f) -> o p f", p=8)
                        )
                        r4 = den.tile([8, 64], F32, tag="r4")
                        nc.vector.reciprocal_approx_fast(r4[:], d4[:])
                        rrow = den.tile([1, 512], F32, tag="rrow")
                        nc.gpsimd.dma_start(
                            rrow.rearrange("o (p f) -> o p f", p=8), r4[:]
                        )
                        den_b = work.tile([64, 512], F32, tag="denb")
                        nc.gpsimd.partition_broadcast(den_b[:], rrow[0:1, :], channels=64)
                        if hh % 2 == 0:
                            nc.vector.tensor_tensor(
                                attnT[0:64, a, qsl],
                                avs[0:D, :], den_b[:], op=ALU.mult,
                            )
                        else:
                            tmp = work.tile([64, 512], BF16, tag="atmp")
                            nc.vector.tensor_tensor(tmp[:], avs[0:D, :], den_b[:], op=ALU.mult)
                            nc.gpsimd.dma_start(attnT[64:128, a, qsl], tmp[:])

            def wo1_rs_thunks(qt, use_act=False):
                # wo1 in [E, token] orientation: the RS output needs no
                # transposes on the consumer side
                o1t = big2.tile([128, 4, 512], BF16, tag="o1t", name=f"o1t{qt}")
                thunks = []

                def mk_ec(ec):
                    def f():
                        ps = psC.tile([128, 512], F32, tag="psC")
                        for ko in range(2):
                            nc.tensor.matmul(
                                ps[:],
                                lhsT=wo1_sb[:, ko, ec * 128:(ec + 1) * 128],
                                rhs=attnT[:, ko, qt * 512:(qt + 1) * 512],
                                start=(ko == 0), stop=(ko == 1),
                            )
                        if use_act:   # last block: ACT is idle, DVE backed up
                            nc.scalar.copy(o1t[:, ec, :], ps[:])
                        else:
                            nc.vector.tensor_copy(o1t[:, ec, :], ps[:])
                    return f

                for ec in range(4):
                    thunks.append(mk_ec(ec))

                def g():
                    # gpsimd queue: the sync queue may be blocked for ~20us
                    # on an RS-waiting o1rsb load, which must not delay the
                    # stores that gate the next RS trigger
                    for hf in range(2):
                        nc.gpsimd.dma_start(
                            o1T_d[qt, hf * E * 256:(hf + 1) * E * 256].rearrange(
                                "(k p t) -> p k t", p=128, t=256
                            ),
                            o1t[:, :, hf * 256:(hf + 1) * 256],
                        )
                    nc.gpsimd.collective_compute(
                        "ReduceScatter", ALU.add, replica_groups=RG_PAIRS,
                        ins=[o1T_d[qt].opt()], outs=[o1rT_d[qt].opt()],
                    )
                thunks.append(g)
                return thunks

            h2map = {}

            def post_rs_thunks(b, dve_ln=False):
                o1rsb = den.tile([128, 4, 256], BF16, tag="o1rsb",
                                 name=f"o1rsb{b}")
                thunks = [lambda: nc.sync.dma_start(
                    o1rsb[:], o1rT_d[b].rearrange("(k p t) -> p k t", p=128, t=256)
                )]

                def mk_wo2(tch, tt):
                    def f():
                        ps = psC.tile([128, 512], F32, tag="psC")
                        for ko in range(4):
                            nc.tensor.matmul(
                                ps[:],
                                lhsT=o1rsb[:, ko, tch * 128:(tch + 1) * 128],
                                rhs=wo2_sb[:, ko, :],
                                start=(ko == 0), stop=(ko == 3),
                            )
                        h2r = work.tile([128, E], F32, tag="wf32", name=f"h2r{tt}")
                        nc.vector.tensor_add(h2r[:], ps[:], xq_sb[:, tt, :])
                        h2map[tt] = h2r
                    return f

                def mk_ln2tp(tt):
                    def g():
                        h2r = h2map.pop(tt)
                        h2t = lnw.tile([128, E], BF16, tag="lnt", name=f"h2t{tt}")
                        # ACT rsqrt where the exp set is still resident (the
                        # gelu switch happens after tile 5); the last block's
                        # tiles use the DVE so they never touch ACT tables
                        if dve_ln:
                            layernorm_tile_dve(h2r[:], h2t)
                        else:
                            layernorm_tile(h2r[:], h2t)
                        for ko in range(4):
                            psT = psC.tile([128, 128], BF16, tag="psC",
                                           name=f"psG{tt}_{ko}")
                            nc.tensor.transpose(
                                psT[:], h2t[:, ko * 128:(ko + 1) * 128], ident[:]
                            )
                            nc.vector.tensor_copy(
                                h2T[:, ko, tt * 128:(tt + 1) * 128], psT[:]
                            )
                    return g

                for tch in range(2):
                    tt = 2 * b + tch
                    thunks.append(mk_wo2(tch, tt))
                    thunks.append(mk_ln2tp(tt))
                return thunks

            m1T_tiles = [
                acts.tile([128, 16, 512], BF16, tag="m1T", name=f"m1T{h}")
                for h in range(2)
            ]

            def mlp_m1(half):
                hsl = slice(half * 512, (half + 1) * 512)
                m1T = m1T_tiles[half]
                for mi in range(16):
                    ps = psC.tile([128, 512], F32, tag="psC")
                    for ko in range(4):
                        nc.tensor.matmul(
                            ps[:],
                            lhsT=w1_sb[:, ko, mi * 128:(mi + 1) * 128],
                            rhs=h2T[:, ko, hsl],
                            start=(ko == 0), stop=(ko == 3),
                        )
                    nc.scalar.activation(m1T[:, mi, :], ps[:], AF.Gelu_apprx_tanh)

            def mlp_m2(half):
                m1T = m1T_tiles[half]
                for tt in range(4 * half, 4 * half + 4):
                    lt = tt % 4
                    ps = psC.tile([128, 512], F32, tag="psC")
                    for ko in range(16):
                        nc.tensor.matmul(
                            ps[:],
                            lhsT=m1T[:, ko, lt * 128:(lt + 1) * 128],
                            rhs=w2_sb[:, ko, :],
                            start=(ko == 0), stop=(ko == 15),
                        )
                    ot = work.tile([128, E], F32, tag="wf32")
                    nc.vector.tensor_add(ot[:], ps[:], xq_sb[:, tt, :])
                    # gpsimd queue: idle at the tail, and the sync queue may
                    # be blocked waiting on RS(3) for the o1rsb load
                    nc.gpsimd.dma_start(out[tt * 128:(tt + 1) * 128, :], ot[:])

            # bulky tail-phase loads issue after the QKV section so the
            # early gpsimd DMA ring stays clear for phase-A-critical traffic
            w1_sb = load_const([128, 4, 4 * E], w1, "w1_sb")
            w2_sb = load_const([128, 16, E], w2, "w2_sb")
            nc.gpsimd.dma_start(xq_sb[:], xq[:])

            # collective warmup: late enough that its global ordering fence
            # doesn't stall the startup DMAs, early enough to finish (~10us)
            # well before RS(0)
            nc.gpsimd.collective_compute(
                "ReduceScatter", ALU.add, replica_groups=RG_PAIRS,
                ins=[warm_in[:].opt()], outs=[warm_out[:].opt()],
            )

            for qt in range(QTS):
                attention_block(qt)
                inject_q.extend(wo1_rs_thunks(qt, use_act=(qt == 3)))
                if qt >= 1:          # drained late in the NEXT block, by
                    inject_q2.extend(post_rs_thunks(qt - 1))   # when RS is done
            while inject_q:          # wo1(3)+RS(3)
                inject_q.popleft()()
            mlp_m1(0)                # dense PE work while RS(2)/RS(3) land
            while inject_q2:         # post_rs(2) pieces
                inject_q2.popleft()()
            for th in post_rs_thunks(3, dve_ln=True):
                th()
            mlp_m2(0)
            mlp_m1(1)
            mlp_m2(1)

    nc.finalize()
    return nc


@functools.lru_cache(maxsize=1)
def _get_graph():
    return _build_graph()


def _bf16_kpm(a, p=128):
    """[K, M] fp32 -> contiguous [p, K//p, M] bf16 (SBUF (k p) layout)."""
    k, m = a.shape
    return np.ascontiguousarray(
        a.reshape(k // p, p, m).transpose(1, 0, 2)
    ).astype(ml_dtypes.bfloat16)


def _f8_kpm(a, p=128):
    k, m = a.shape
    return np.ascontiguousarray(
        a.reshape(k // p, p, m).transpose(1, 0, 2)
    ).astype(ml_dtypes.float8_e4m3)


def _own_rows(rank):
    """Global row indices owned by a core after the per-block reduce-scatters."""
    return np.concatenate(
        [np.arange(512 * qt + 256 * rank, 512 * qt + 256 * rank + 256) for qt in range(QTS)]
    )


def _make_in_maps(x, wq, wk, wv, wo1, wo2, w1, w2, ln1_scale, ln2_scale):
    x = np.asarray(x, dtype=np.float32)
    wq = np.asarray(wq, dtype=np.float32).reshape(E, H * D)
    wk = np.asarray(wk, dtype=np.float32).reshape(E, H * D)
    wv = np.asarray(wv, dtype=np.float32).reshape(E, H * D)
    wo1 = np.asarray(wo1, dtype=np.float32).reshape(H * D, E)
    wo2 = np.asarray(wo2, dtype=np.float32)
    w1 = np.asarray(w1, dtype=np.float32)
    w2 = np.asarray(w2, dtype=np.float32)
    s1 = np.asarray(ln1_scale, dtype=np.float32)[:, None]
    s2 = np.asarray(ln2_scale, dtype=np.float32)[:, None]

    wq_s, wk_s, wv_s = s1 * wq, s1 * wk, s1 * wv
    w1_s = s2 * w1

    # causal mask patterns for diagonal 128-chunks within a 512 q-tile:
    # mask_j[p, f] = 1.0 iff (128j + p) <= f;  stored [p, j, f]
    iota_p = np.arange(128)[:, None]
    iota_f = np.arange(512)[None, :]
    mask_np = np.ascontiguousarray(np.stack(
        [(128 * j + iota_p <= iota_f).astype(np.float32) for j in range(4)]
    ).transpose(1, 0, 2)).astype(ml_dtypes.bfloat16)

    in_maps = []
    for c in range(8):
        b, g = c // 2, c % 2
        hd = slice(g * HG * D, (g + 1) * HG * D)
        rows = _own_rows(c % 2)
        xq_arr = np.ascontiguousarray(
            x[b][rows].reshape(NTQ, 128, E).transpose(1, 0, 2)
        ).astype(ml_dtypes.bfloat16)
        in_maps.append({
            "xf": np.ascontiguousarray(x[b].reshape(NT, 128, E).transpose(1, 0, 2)).astype(ml_dtypes.bfloat16),
            "xq": xq_arr,
            "wq": _bf16_kpm(wq_s[:, hd]),
            "wk": _bf16_kpm(wk_s[:, hd]),
            "wv": _bf16_kpm(wv_s[:, hd]),
            "wo1": _bf16_kpm(wo1[hd, :]),
            "wo2": _bf16_kpm(wo2),
            "w1": _bf16_kpm(w1_s),
            "w2": _bf16_kpm(w2),
            "masks": mask_np,
        })
    return in_maps


def run(trace=False, **inputs):
    nc = _get_graph()
    in_maps = _make_in_maps(**inputs)
    res = run_bass_kernel_spmd(nc, in_maps, core_ids=list(range(8)), trace=trace)
    y = np.empty((B, S, E), dtype=np.float32)
    for c in range(8):
        b = c // 2
        y[b][_own_rows(c % 2)] = res.results[c]["out"]
    return y, res


def kernel(**inputs):
    y, _ = run(trace=False, **inputs)
    return y


# revision 48
# speedup vs baseline: 1.0073x; 1.0073x over previous
"""Trainium2 Bass kernel for nn_Block_54382875902076 (dense transformer block).

Reference computation (B=4, S=2048, E=512, H=8, D=64, fp32):
    res = x
    h   = LN1(x)                      (no bias, eps=1e-6)
    h   = res + Attn(h)               (causal, wo1 [H,D,E] then wo2 [E,E])
    h   = LN2(h)
    out = res + gelu(h @ w1) @ w2     (NOTE: res = block input, both residuals)

Sharding (8 cores): core c = (batch b = c//2, head-group g = c%2).
Each core computes LN1 + QKV for its 4 heads over the full sequence,
exact-causal attention, the wo1 partial projection in [E, token]
orientation, then pair-wise ReduceScatters sum the two head-groups'
partials and hand each core half of every block's rows (already
E-major, no transposes needed) for wo2 + LN2 + MLP.

Schedule (HW-measured ~281 us, from a 336 us starting point):
 - the attention exp stream on the ACT engine is the spine (~1 us per
   128x1024 chunk; diagonal chunks exp/mask/AV only the causally-valid
   column range); score matmuls run one chunk ahead of the AV matmuls
   so the PE never waits on the exp
 - wo1/RS work drains at the front of the next attention block and
   post-RS wo2/LN2/transpose work drains in its last chunks (the RS
   itself takes 10-20 us), one piece per chunk in the PE's idle slots
 - a full-size warmup ReduceScatter pays the collective channel setup
   (~25 us) off the critical path AND pins the channel to the fast
   hardware-descriptor mode (a small warmup pins it to the slow
   software path: 21 GB/s vs 50 GB/s measured)
 - DMA queues are balanced so nothing phase-critical queues behind a
   blocked wait: bulky w1/w2/xq loads issue after the QKV section,
   masks ride the scalar ring, out-stores ride gpsimd (sync may be
   blocked waiting on RS(3) for the last o1rsb load)
 - LN2 uses ACT's ln/exp rsqrt for tiles 0-5 (before the single gelu
   table switch) and a DVE bit-trick rsqrt (magic seed + 2 Newton
   steps) for the RS(3)-gated tiles 6-7, so the ACT table set never
   thrashes between exp and gelu
 - fp8 DoubleRow for the MLP was tried and REVERTED: e4m3's 3-bit
   mantissa costs ~3e-2 rel L2 on this problem (gate is 2e-2), even
   with range-normalizing scales; numpy-simulated error matched HW
   exactly, so the quantization itself is the floor, not a bug
"""

import functools
import sys
from collections import deque

import numpy as np

for _p in ("/opt/trn_rl_repo", "/root/.axon_site/_ro/trn_rl_repo"):
    if _p not in sys.path:
        sys.path.append(_p)

import ml_dtypes  # noqa: E402
import concourse.bass as bass  # noqa: E402
import concourse.tile as tile  # noqa: E402
from concourse import bacc, mybir  # noqa: E402
from concourse.bass_utils import run_bass_kernel_spmd  # noqa: E402

_ALLOWED_ACT_SETS = {"natural_log_exp_and_others", "gelu_apprx_tanh_and_others"}
_orig_get_act_tables = bacc.get_activation_tables


def _filtered_act_tables(module_arch):
    tabs = _orig_get_act_tables(module_arch)
    return {
        name: (funcs if name in _ALLOWED_ACT_SETS else set())
        for name, funcs in tabs.items()
    }


bacc.get_activation_tables = _filtered_act_tables

F32 = mybir.dt.float32
I32 = mybir.dt.int32
BF16 = mybir.dt.bfloat16
F8 = mybir.dt.float8e4
DR = mybir.MatmulPerfMode.DoubleRow
AF = mybir.ActivationFunctionType
ALU = mybir.AluOpType

B, S, E, H, D = 4, 2048, 512, 8, 64
HG = H // 2            # heads per core
SQ = S // 2            # rows per core after reduce-scatter
NT = S // 128          # 16 token tiles (full seq)
NTQ = SQ // 128        # 8 token tiles (own half)
QTS = S // 512         # 4 q-tiles of 512 for attention
RG_PAIRS = [[0, 1], [2, 3], [4, 5], [6, 7]]


def _build_graph():
    nc = bacc.Bacc("TRN2", target_bir_lowering=False, debug=False, num_devices=8)

    xf = nc.declare_dram_parameter("xf", [128, NT, E], BF16, isOutput=False)
    xq = nc.declare_dram_parameter("xq", [128, NTQ, E], BF16, isOutput=False)
    wq = nc.declare_dram_parameter("wq", [128, 4, HG * D], BF16, isOutput=False)
    wk = nc.declare_dram_parameter("wk", [128, 4, HG * D], BF16, isOutput=False)
    wv = nc.declare_dram_parameter("wv", [128, 4, HG * D], BF16, isOutput=False)
    wo1 = nc.declare_dram_parameter("wo1", [128, 2, E], BF16, isOutput=False)
    wo2 = nc.declare_dram_parameter("wo2", [128, 4, E], BF16, isOutput=False)
    w1 = nc.declare_dram_parameter("w1", [128, 4, 4 * E], BF16, isOutput=False)
    w2 = nc.declare_dram_parameter("w2", [128, 16, E], BF16, isOutput=False)
    masks = nc.declare_dram_parameter("masks", [128, 4, 512], BF16, isOutput=False)
    out = nc.declare_dram_parameter("out", [SQ, E], F32, isOutput=True)

    with tile.TileContext(nc) as tc:
        with (
            tc.tile_pool(name="consts", bufs=1) as consts,
            tc.tile_pool(name="acts", bufs=1) as acts,
            tc.tile_pool(name="work", bufs=3) as work,
            tc.tile_pool(name="big2", bufs=2) as big2,
            tc.tile_pool(name="stats", bufs=6) as stats,
            tc.tile_pool(name="den", bufs=2) as den,
            tc.tile_pool(name="lnw", bufs=5) as lnw,
            tc.tile_pool(name="expp", bufs=3) as expp,
            tc.tile_pool(name="psA", bufs=2, space="PSUM") as psA,
            tc.tile_pool(name="psB", bufs=2, space="PSUM") as psB,
            tc.tile_pool(name="psC", bufs=2, space="PSUM") as psC,
            tc.tile_pool(name="dram", bufs=1, space="DRAM") as dram,
        ):
            # warmup buffers shaped exactly like the real RS: the first
            # collective fixes the channel's descriptor mode, and a tiny
            # warmup would pin it to the slow software-descriptor path
            warm_in = dram.tile([2 * E * 256], BF16, name="warm_in")
            warm_out = dram.tile([E * 256], BF16, name="warm_out")
            # flat per-block buffers: a fully-contiguous 1D AP keeps the
            # collective's DMA on the fast hardware-descriptor path
            o1T_d = dram.tile([QTS, 2 * E * 256], BF16, name="o1T_d")
            o1rT_d = dram.tile([QTS, E * 256], BF16, name="o1rT_d")

            # ---- constants / weights (contiguous loads, gpsimd queue) ----
            eps_t = consts.tile([128, 1], F32)
            nc.vector.memset(eps_t, 1e-6)
            magic_t = consts.tile([128, 1], I32, tag="magic")
            nc.vector.memset(magic_t, 0x5F3759DF)
            ident = consts.tile([128, 128], BF16)
            from concourse.masks import make_identity
            make_identity(nc, ident[:])

            def load_const(shape, src, tag, dtype=BF16):
                t = consts.tile(shape, dtype, tag=tag)
                nc.gpsimd.dma_start(t[:], src[:])
                return t

            wq_sb = load_const([128, 4, HG * D], wq, "wq_sb")
            wk_sb = load_const([128, 4, HG * D], wk, "wk_sb")
            wv_sb = load_const([128, 4, HG * D], wv, "wv_sb")
            wo1_sb = load_const([128, 2, E], wo1, "wo1_sb")
            wo2_sb = load_const([128, 4, E], wo2, "wo2_sb")
            masks_sb = consts.tile([128, 4, 512], BF16, tag="masks_sb")
            nc.scalar.dma_start(masks_sb[:], masks[:])
            xq_sb = acts.tile([128, NTQ, E], BF16)

            def layernorm_tile(src_ap, dst_tile):
                """dst (bf16) = (src - mean) * rsqrt(var + eps); ACT rsqrt."""
                st6 = stats.tile([128, 6], F32, tag="st6")
                nc.vector.bn_stats(st6[:], src_ap)
                mv = stats.tile([128, 2], F32, tag="mv")
                nc.vector.bn_aggr(mv[:], st6[:])
                lnv = stats.tile([128, 1], F32, tag="lnv")
                nc.scalar.activation(lnv[:], mv[:, 1:2], AF.Ln, bias=eps_t[:])
                rsig = stats.tile([128, 1], F32, tag="rsig")
                nc.scalar.activation(rsig[:], lnv[:], AF.Exp, scale=-0.5)
                nc.vector.tensor_scalar(
                    dst_tile[:], src_ap, mv[:, 0:1], rsig[:],
                    op0=ALU.subtract, op1=ALU.mult,
                )

            def layernorm_tile_dve(src_ap, dst_tile):
                """Same, but rsqrt on DVE (bit-trick seed + 2 Newton steps)
                so the ACT table set is untouched (no exp/gelu thrash)."""
                st6 = stats.tile([128, 6], F32, tag="st6")
                nc.vector.bn_stats(st6[:], src_ap)
                mv = stats.tile([128, 2], F32, tag="mv")
                nc.vector.bn_aggr(mv[:], st6[:])
                v = mv[:, 1:2]
                y = stats.tile([128, 1], F32, tag="yrs")
                iv = stats.tile([128, 1], I32, tag="ivrs")
                nc.vector.tensor_scalar(
                    iv[:], v.bitcast(I32), 1, None, op0=ALU.arith_shift_right
                )
                nc.vector.tensor_tensor(
                    y.bitcast(I32)[:], magic_t[:], iv[:], op=ALU.subtract
                )
                for it in range(2):
                    y2 = stats.tile([128, 1], F32, tag="y2rs")
                    nc.vector.tensor_tensor(y2[:], y[:], y[:], op=ALU.mult)
                    t = stats.tile([128, 1], F32, tag="trs")
                    nc.vector.tensor_tensor(t[:], y2[:], v, op=ALU.mult)
                    t2 = stats.tile([128, 1], F32, tag="t2rs")
                    nc.vector.tensor_scalar(
                        t2[:], t[:], -0.5, 1.5, op0=ALU.mult, op1=ALU.add
                    )
                    yn = stats.tile([128, 1], F32, tag="ynrs")
                    nc.vector.tensor_tensor(yn[:], y[:], t2[:], op=ALU.mult)
                    y = yn
                nc.vector.tensor_scalar(
                    dst_tile[:], src_ap, mv[:, 0:1], y[:],
                    op0=ALU.subtract, op1=ALU.mult,
                )

            # ---- LN1-folded QKV from host-transposed x -------------------
            # K = rsig*(wk^T x - mu*colsum(wk)) etc: the projections consume
            # xT directly (no h1 transposes); the mean correction is a rank-1
            # accumulate and rsig rides a broadcast multiply (K/Q) or the ACT
            # copy's per-partition scale (V)
            # ---- LN1 + per-block transpose + QKV -------------------------
            xfsb = consts.tile([128, NT, E], BF16, tag="xfsb")
            for _st in range(QTS):
                nc.sync.dma_start(
                    xfsb[:, 4 * _st:4 * _st + 4, :], xf[:, 4 * _st:4 * _st + 4, :]
                )
            h1T = acts.tile([128, 4, S], BF16)
            KT = acts.tile([128, 2, S], BF16)
            QT = acts.tile([128, 2, S], BF16)
            V65 = acts.tile([128, NT, HG, D + 1], BF16)
            nc.vector.memset(V65[:, :, :, D:D + 1], 1.0)
            for st in range(4):
                h1ts0 = []
                for t in range(4 * st, 4 * st + 4):
                    h1t = lnw.tile([128, E], BF16, tag="lnt", name=f"h1t{t}")
                    layernorm_tile(xfsb[:, t, :], h1t)
                    h1ts0.append(h1t)
                for lt in range(4):
                    for ko in range(4):
                        psT = psC.tile([128, 128], BF16, tag="psC",
                                       name=f"psH{st}_{lt}_{ko}")
                        nc.tensor.transpose(
                            psT[:], h1ts0[lt][:, ko * 128:(ko + 1) * 128], ident[:]
                        )
                        nc.vector.tensor_copy(
                            h1T[:, ko, st * 512 + lt * 128:st * 512 + (lt + 1) * 128],
                            psT[:],
                        )
                sl = slice(st * 512, (st + 1) * 512)
                for mi in range(2):
                    for dst, w_sb in ((KT, wk_sb), (QT, wq_sb)):
                        ps = psC.tile([128, 512], F32, tag="psC")
                        for ko in range(4):
                            nc.tensor.matmul(
                                ps[:],
                                lhsT=w_sb[:, ko, mi * 128:(mi + 1) * 128],
                                rhs=h1T[:, ko, sl],
                                start=(ko == 0), stop=(ko == 3),
                            )
                        # ACT is idle in this phase; the PSUM->SBUF casts are
                        # the DVE's biggest load here, so move them over
                        nc.scalar.copy(dst[:, mi, sl], ps[:])
                for tt in range(4 * st, 4 * st + 4):
                    ps = psC.tile([128, 512], F32, tag="psC")
                    for ko in range(4):
                        nc.tensor.matmul(
                            ps[:, 0:HG * D],
                            lhsT=h1T[:, ko, tt * 128:(tt + 1) * 128],
                            rhs=wv_sb[:, ko, :],
                            start=(ko == 0), stop=(ko == 3),
                        )
                    nc.scalar.copy(
                        V65[:, tt, :, 0:D],
                        ps[:, 0:HG * D].rearrange("p (h d) -> p h d", h=HG),
                    )

            # ---- causal attention spine + injected wo1/RS/wo2/LN2 --------
            attnT = acts.tile([128, 2, S], BF16)
            h2T = acts.tile([128, 4, SQ], BF16)
            inject_q = deque()    # wo1 pieces: drained front-of-block
            inject_q2 = deque()   # post-RS pieces: drained in late chunks

            def attention_block(qt):
                ext = 4 * (qt + 1)           # causal extent in 128-chunks
                qsl = slice(qt * 512, (qt + 1) * 512)
                for a in range(2):           # local head pairs (2a, 2a+1)
                    avA = psB.tile([D + 1, 512], F32, tag="psB")
                    avB = psB.tile([D + 1, 512], F32, tag="psB")

                    def scores(c):
                        # the two 64-row matmuls auto-pack into disjoint PE
                        # row-bands (tile_position from base_partition) and
                        # stream concurrently
                        sp = psA.tile([128, 1024], F32, tag="psA")
                        nc.tensor.matmul(
                            sp[:, 0:512],
                            lhsT=KT[0:64, a, c * 128:(c + 1) * 128],
                            rhs=QT[0:64, a, qsl],
                            start=True, stop=True,
                        )
                        nc.tensor.matmul(
                            sp[:, 512:1024],
                            lhsT=KT[64:128, a, c * 128:(c + 1) * 128],
                            rhs=QT[64:128, a, qsl],
                            start=True, stop=True,
                        )
                        return sp

                    sps = {0: scores(0)}
                    for c in range(ext):
                        sp = sps.pop(c)
                        j = c - 4 * qt
                        # diagonal chunk j: columns < 128j have no valid keys
                        # in this chunk, so exp/mask/AV skip them entirely
                        lo = 128 * j if j > 0 else 0
                        ex = expp.tile([128, 2, 512], BF16, tag="ex")
                        nc.scalar.activation(
                            ex[:, :, lo:512],
                            sp.rearrange("p (h f) -> p h f", h=2)[:, :, lo:512],
                            AF.Exp, scale=D ** -0.5,
                        )
                        if c + 1 < ext:      # scores one chunk ahead of AV
                            sps[c + 1] = scores(c + 1)
                        if j >= 0:           # diagonal chunk: causal mask
                            for half in range(2):
                                exh = ex[:, half, lo:512]
                                nc.vector.tensor_mul(
                                    exh, exh, masks_sb[:, j, lo:512]
                                )
                        nc.tensor.matmul(
                            avA[:, lo:512], lhsT=V65[:, c, 2 * a, :],
                            rhs=ex[:, 0, lo:512],
                            start=(c == 0), stop=(c == ext - 1),
                            skip_group_check=True,
                        )
                        nc.tensor.matmul(
                            avB[:, lo:512], lhsT=V65[:, c, 2 * a + 1, :],
                            rhs=ex[:, 1, lo:512],
                            start=(c == 0), stop=(c == ext - 1),
                            skip_group_check=True,
                        )
                        if a == 1 and c >= ext - 8 and inject_q2:
                            inject_q2.popleft()()
                        elif inject_q:
                            inject_q.popleft()()
                    for hh, av in ((2 * a, avA), (2 * a + 1, avB)):
                        # softmax denominator rides as the ones-row (row 64);
                        # reciprocal runs [8,64] (DVE recip cost scales with
                        # free size per lane) via a DMA scatter/gather pair
                        avs = work.tile([D + 1, 512], F32, tag="avs")
                        nc.vector.tensor_copy(avs[:], av[:])
                        d4 = den.tile([8, 64], F32, tag="d4")
                        nc.gpsimd.dma_start(
                            d4[:], avs[D:D + 1, :].rearrange("o (p f) -> o p f", p=8)
                        )
                        r4 = den.tile([8, 64], F32, tag="r4")
                        nc.vector.reciprocal_approx_fast(r4[:], d4[:])
                        rrow = den.tile([1, 512], F32, tag="rrow")
                        nc.gpsimd.dma_start(
                            rrow.rearrange("o (p f) -> o p f", p=8), r4[:]
                        )
                        den_b = work.tile([64, 512], F32, tag="denb")
                        nc.gpsimd.partition_broadcast(den_b[:], rrow[0:1, :], channels=64)
                        if hh % 2 == 0:
                            nc.vector.tensor_tensor(
                                attnT[0:64, a, qsl],
                                avs[0:D, :], den_b[:], op=ALU.mult,
                            )
                        else:
                            tmp = work.tile([64, 512], BF16, tag="atmp")
                            nc.vector.tensor_tensor(tmp[:], avs[0:D, :], den_b[:], op=ALU.mult)
                            nc.gpsimd.dma_start(attnT[64:128, a, qsl], tmp[:])

            def wo1_rs_thunks(qt, use_act=False):
                # wo1 in [E, token] orientation: the RS output needs no
                # transposes on the consumer side
                o1t = big2.tile([128, 4, 512], BF16, tag="o1t", name=f"o1t{qt}")
                thunks = []

                def mk_ec(ec):
                    def f():
                        ps = psC.tile([128, 512], F32, tag="psC")
                        for ko in range(2):
                            nc.tensor.matmul(
                                ps[:],
                                lhsT=wo1_sb[:, ko, ec * 128:(ec + 1) * 128],
                                rhs=attnT[:, ko, qt * 512:(qt + 1) * 512],
                                start=(ko == 0), stop=(ko == 1),
                            )
                        if use_act:   # last block: ACT is idle, DVE backed up
                            nc.scalar.copy(o1t[:, ec, :], ps[:])
                        else:
                            nc.vector.tensor_copy(o1t[:, ec, :], ps[:])
                    return f

                for ec in range(4):
                    thunks.append(mk_ec(ec))

                def g():
                    # gpsimd queue: the sync queue may be blocked for ~20us
                    # on an RS-waiting o1rsb load, which must not delay the
                    # stores that gate the next RS trigger
                    for hf in range(2):
                        nc.gpsimd.dma_start(
                            o1T_d[qt, hf * E * 256:(hf + 1) * E * 256].rearrange(
                                "(k p t) -> p k t", p=128, t=256
                            ),
                            o1t[:, :, hf * 256:(hf + 1) * 256],
                        )
                    nc.gpsimd.collective_compute(
                        "ReduceScatter", ALU.add, replica_groups=RG_PAIRS,
                        ins=[o1T_d[qt].opt()], outs=[o1rT_d[qt].opt()],
                    )
                thunks.append(g)
                return thunks

            h2map = {}

            def post_rs_thunks(b, dve_ln=False):
                o1rsb = den.tile([128, 4, 256], BF16, tag="o1rsb",
                                 name=f"o1rsb{b}")
                thunks = [lambda: nc.sync.dma_start(
                    o1rsb[:], o1rT_d[b].rearrange("(k p t) -> p k t", p=128, t=256)
                )]

                def mk_wo2(tch, tt):
                    def f():
                        ps = psC.tile([128, 512], F32, tag="psC")
                        for ko in range(4):
                            nc.tensor.matmul(
                                ps[:],
                                lhsT=o1rsb[:, ko, tch * 128:(tch + 1) * 128],
                                rhs=wo2_sb[:, ko, :],
                                start=(ko == 0), stop=(ko == 3),
                            )
                        h2r = work.tile([128, E], F32, tag="wf32", name=f"h2r{tt}")
                        nc.vector.tensor_add(h2r[:], ps[:], xq_sb[:, tt, :])
                        h2map[tt] = h2r
                    return f

                def mk_ln2tp(tt):
                    def g():
                        h2r = h2map.pop(tt)
                        h2t = lnw.tile([128, E], BF16, tag="lnt", name=f"h2t{tt}")
                        # ACT rsqrt where the exp set is still resident (the
                        # gelu switch happens after tile 5); the last block's
                        # tiles use the DVE so they never touch ACT tables
                        if dve_ln:
                            layernorm_tile_dve(h2r[:], h2t)
                        else:
                            layernorm_tile(h2r[:], h2t)
                        for ko in range(4):
                            psT = psC.tile([128, 128], BF16, tag="psC",
                                           name=f"psG{tt}_{ko}")
                            nc.tensor.transpose(
                                psT[:], h2t[:, ko * 128:(ko + 1) * 128], ident[:]
                            )
                            nc.vector.tensor_copy(
                                h2T[:, ko, tt * 128:(tt + 1) * 128], psT[:]
                            )
                    return g

                for tch in range(2):
                    tt = 2 * b + tch
                    thunks.append(mk_wo2(tch, tt))
                    thunks.append(mk_ln2tp(tt))
                return thunks

            m1T_tiles = [
                acts.tile([128, 16, 512], BF16, tag="m1T", name=f"m1T{h}")
                for h in range(2)
            ]

            def mlp_m1(half):
                hsl = slice(half * 512, (half + 1) * 512)
                m1T = m1T_tiles[half]
                for mi in range(16):
                    ps = psC.tile([128, 512], F32, tag="psC")
                    for ko in range(4):
                        nc.tensor.matmul(
                            ps[:],
                            lhsT=w1_sb[:, ko, mi * 128:(mi + 1) * 128],
                            rhs=h2T[:, ko, hsl],
                            start=(ko == 0), stop=(ko == 3),
                        )
                    nc.scalar.activation(m1T[:, mi, :], ps[:], AF.Gelu_apprx_tanh)

            def mlp_m2(half):
                m1T = m1T_tiles[half]
                for tt in range(4 * half, 4 * half + 4):
                    lt = tt % 4
                    ps = psC.tile([128, 512], F32, tag="psC")
                    for ko in range(16):
                        nc.tensor.matmul(
                            ps[:],
                            lhsT=m1T[:, ko, lt * 128:(lt + 1) * 128],
                            rhs=w2_sb[:, ko, :],
                            start=(ko == 0), stop=(ko == 15),
                        )
                    ot = work.tile([128, E], F32, tag="wf32")
                    nc.vector.tensor_add(ot[:], ps[:], xq_sb[:, tt, :])
                    # gpsimd queue: idle at the tail, and the sync queue may
                    # be blocked waiting on RS(3) for the o1rsb load
                    nc.gpsimd.dma_start(out[tt * 128:(tt + 1) * 128, :], ot[:])

            # bulky tail-phase loads issue after the QKV section so the
            # early gpsimd DMA ring stays clear for phase-A-critical traffic
            w1_sb = load_const([128, 4, 4 * E], w1, "w1_sb")
            w2_sb = load_const([128, 16, E], w2, "w2_sb")
            nc.gpsimd.dma_start(xq_sb[:], xq[:])

            # collective warmup: late enough that its global ordering fence
            # doesn't stall the startup DMAs, early enough to finish (~10us)
            # well before RS(0)
            nc.gpsimd.collective_compute(
                "ReduceScatter", ALU.add, replica_groups=RG_PAIRS,
                ins=[warm_in[:].opt()], outs=[warm_out[:].opt()],
            )

            for qt in range(QTS):
                attention_block(qt)
                inject_q.extend(wo1_rs_thunks(qt, use_act=(qt == 3)))
                if qt >= 1:          # drained late in the NEXT block, by
                    # block 2's pieces run after m1(0)'s gelus, so its LN2
                    # must use the DVE rsqrt to keep the gelu table resident
                    inject_q2.extend(post_rs_thunks(qt - 1, dve_ln=(qt == 3)))
            while inject_q:          # wo1(3)+RS(3)
                inject_q.popleft()()
            mlp_m1(0)                # dense PE work while RS(2)/RS(3) land
            while inject_q2:         # post_rs(2) pieces
                inject_q2.popleft()()
            for th in post_rs_thunks(3, dve_ln=True):
                th()
            mlp_m2(0)
            mlp_m1(1)
            mlp_m2(1)

    nc.finalize()
    return nc


@functools.lru_cache(maxsize=1)
def _get_graph():
    return _build_graph()


def _bf16_kpm(a, p=128):
    """[K, M] fp32 -> contiguous [p, K//p, M] bf16 (SBUF (k p) layout)."""
    k, m = a.shape
    return np.ascontiguousarray(
        a.reshape(k // p, p, m).transpose(1, 0, 2)
    ).astype(ml_dtypes.bfloat16)


def _f8_kpm(a, p=128):
    k, m = a.shape
    return np.ascontiguousarray(
        a.reshape(k // p, p, m).transpose(1, 0, 2)
    ).astype(ml_dtypes.float8_e4m3)


def _own_rows(rank):
    """Global row indices owned by a core after the per-block reduce-scatters."""
    return np.concatenate(
        [np.arange(512 * qt + 256 * rank, 512 * qt + 256 * rank + 256) for qt in range(QTS)]
    )


def _make_in_maps(x, wq, wk, wv, wo1, wo2, w1, w2, ln1_scale, ln2_scale):
    x = np.asarray(x, dtype=np.float32)
    wq = np.asarray(wq, dtype=np.float32).reshape(E, H * D)
    wk = np.asarray(wk, dtype=np.float32).reshape(E, H * D)
    wv = np.asarray(wv, dtype=np.float32).reshape(E, H * D)
    wo1 = np.asarray(wo1, dtype=np.float32).reshape(H * D, E)
    wo2 = np.asarray(wo2, dtype=np.float32)
    w1 = np.asarray(w1, dtype=np.float32)
    w2 = np.asarray(w2, dtype=np.float32)
    s1 = np.asarray(ln1_scale, dtype=np.float32)[:, None]
    s2 = np.asarray(ln2_scale, dtype=np.float32)[:, None]

    wq_s, wk_s, wv_s = s1 * wq, s1 * wk, s1 * wv
    w1_s = s2 * w1

    # causal mask patterns for diagonal 128-chunks within a 512 q-tile:
    # mask_j[p, f] = 1.0 iff (128j + p) <= f;  stored [p, j, f]
    iota_p = np.arange(128)[:, None]
    iota_f = np.arange(512)[None, :]
    mask_np = np.ascontiguousarray(np.stack(
        [(128 * j + iota_p <= iota_f).astype(np.float32) for j in range(4)]
    ).transpose(1, 0, 2)).astype(ml_dtypes.bfloat16)

    in_maps = []
    for c in range(8):
        b, g = c // 2, c % 2
        hd = slice(g * HG * D, (g + 1) * HG * D)
        rows = _own_rows(c % 2)
        xq_arr = np.ascontiguousarray(
            x[b][rows].reshape(NTQ, 128, E).transpose(1, 0, 2)
        ).astype(ml_dtypes.bfloat16)
        in_maps.append({
            "xf": np.ascontiguousarray(x[b].reshape(NT, 128, E).transpose(1, 0, 2)).astype(ml_dtypes.bfloat16),
            "xq": xq_arr,
            "wq": _bf16_kpm(wq_s[:, hd]),
            "wk": _bf16_kpm(wk_s[:, hd]),
            "wv": _bf16_kpm(wv_s[:, hd]),
            "wo1": _bf16_kpm(wo1[hd, :]),
            "wo2": _bf16_kpm(wo2),
            "w1": _bf16_kpm(w1_s),
            "w2": _bf16_kpm(w2),
            "masks": mask_np,
        })
    return in_maps


def run(trace=False, **inputs):
    nc = _get_graph()
    in_maps = _make_in_maps(**inputs)
    res = run_bass_kernel_spmd(nc, in_maps, core_ids=list(range(8)), trace=trace)
    y = np.empty((B, S, E), dtype=np.float32)
    for c in range(8):
        b = c // 2
        y[b][_own_rows(c % 2)] = res.results[c]["out"]
    return y, res


def kernel(**inputs):
    y, _ = run(trace=False, **inputs)
    return y
